# revision 1
# baseline (speedup 1.0000x reference)
"""Channel-attention module (CAM) forward for Trainium2.

Computes, per batch b:
    f1 = x[b].reshape(C, H*W)                      # [512, 4096]
    S  = f1 @ f1.T                                 # [512, 512] (symmetric)
    G  = softmax(S_max - S, axis=-1) == exp(S_min_row - S) / rowsum
    fc = G @ f1
    y[b] = beta * fc + x[b]

Sharding: data-parallel over batch B=16 across 8 NeuronCores (2/core).

Key restructuring vs a direct port:
  - Global-shift symmetric exp: E = exp(s0 - S) with one scalar s0
    (global min of S) is symmetric, so fc_raw = E @ f1 takes its
    matmul lhsT directly from E's stored row-tiles (the slice for
    (k-pair, m) is e2[q][:, :, m*128:+128] by symmetry) — no G
    transpose. The per-row softmax scale folds into the epilogue:
    y = (beta / Zraw[c]) * fc_raw + x with Zraw = rowsum(E); the s0
    and row-min shifts cancel exactly.
  - fp8e4 matmuls in DoubleRow perf mode (two 128-deep k-tiles per
    instruction at 0.5 cycles/row) for both S and fc; fp32 PSUM.
  - f1^T built with PE transpose-mode matmuls. TRN2 requires fp8
    transpose outputs at element step 2, so xp/f1t keep fp8 values in
    the low byte of 2-byte cells; each 8-block PSUM bank is drained
    by one dense bitcast-fp16 copy (2x DVE mode) and the S matmuls
    read the even bytes via step-2 APs. No DMA xbar transposes.
    Transpose groups and fc accumulators share one 4-bank PSUM pool
    (single tag); S keeps its own 4 banks. Casts are always emitted
    before the transpose groups that read them (the tile framework
    resolves dependencies in program order).
  - S is emitted k-outer over the first half and m-outer over the
    second so each negated row-min reduce fires as its m-tile stops;
    the global-min chain is DVE reduce -> Pool partition_all_reduce.
    exp on ACT writes E straight to fp8 with the fp32 rowsum accum.
  - x is staged host-side to fp16 (the kernel's ingest precision) and
    y is produced in fp16. beta-robust: Zraw is clamped before the
    reciprocal and br4 is written through a beta!=0-predicated copy,
    so beta=0 yields exactly y = x even if a degenerate row
    overflowed the softmax normalizer.
  - All HBM DMA issues from the otherwise-idle SP sequencer via HWDGE
    at [128,2048] granularity (except the first batch's loads and the
    h1 stores, which split finer to shorten pipeline fill and drain);
    [loads b][loads b+1][stores b] order on the queue. PSUM is only
    readable by ACT/DVE, so the epilogue alternates DVE
    scalar_tensor_tensor with ACT(mul)+DVE/Pool(fp16 add) pairs, and
    the second batch's prep/S/stats chain is emitted with priority
    through the first batch's fc quarters so the terminal
    S->exp->fc->store chain starts as early as possible.
"""

import numpy as np

B, C, HW = 16, 512, 4096
NCORES = 8
BL = B // NCORES  # batches per core
P = 128
CT = C // P       # 4 c-tiles of 128 channels
F = 512           # psum free dim / fc n-chunk
NQ = HW // 4      # 1024: cast/quarter granularity
HALF = HW // 2    # 2048: load/store granularity
KTH = 16          # k-tiles per half

_CACHE = {}
_PHASES = []  # (label, next-instruction marker) for offline timeline analysis


def _build():
    import concourse.bass as bass  # noqa: F401
    import concourse.mybir as mybir
    import concourse.tile as tile
    from concourse import bacc, bass_isa
    from concourse.masks import make_identity

    f32 = mybir.dt.float32
    f16 = mybir.dt.float16
    f8 = mybir.dt.float8e4
    AF = mybir.ActivationFunctionType
    OP = mybir.AluOpType
    AX = mybir.AxisListType
    DR = mybir.MatmulPerfMode.DoubleRow

    nc = bacc.Bacc("TRN2", target_bir_lowering=False, debug=False)
    x_d = nc.dram_tensor("x", [BL, C, HW], f16, kind="ExternalInput")
    beta_d = nc.dram_tensor("beta", [1], f32, kind="ExternalInput")
    y_d = nc.dram_tensor("y", [BL, C, HW], f16, kind="ExternalOutput")

    def _mark(label):
        _PHASES.append((label, nc.get_next_instruction_name()))

    with tile.TileContext(nc) as tc:
        with (
            tc.tile_pool(name="singles", bufs=1) as singles,
            tc.tile_pool(name="x16", bufs=16) as x16_p,     # [128,2048] f16
            tc.tile_pool(name="f8", bufs=4) as f8_p,        # [128,2,4096] f8
            tc.tile_pool(name="f1t", bufs=3) as f1t_p,      # [128,16,512] f16
            tc.tile_pool(name="e2", bufs=4) as e2_p,        # [128,2,512] f8
            tc.tile_pool(name="soft", bufs=28) as soft_p,   # [128,<=4] f32
            tc.tile_pool(name="outs", bufs=12) as out_p,
            tc.tile_pool(name="tmps", bufs=6) as tmp_p,    # [128,2048] f16
            tc.tile_pool(name="ps_s", bufs=4, space="PSUM") as ps_s,
            tc.tile_pool(name="ps_mix", bufs=4, space="PSUM") as ps_mix,
        ):
            beta_sb = singles.tile([P, 1], f32)
            nc.sync.dma_start(out=beta_sb[:], in_=beta_d[:].to_broadcast([P, 1]))
            ident = singles.tile([P, P], f8)
            make_identity(nc, ident[:])
            beta_nz = singles.tile([P, 1], mybir.dt.uint8)
            nc.vector.tensor_scalar(
                out=beta_nz[:], in0=beta_sb[:], scalar1=0.0, scalar2=None,
                op0=OP.not_equal,
            )

            state = {}

            def emit_loads(b, fine_h0):
                _mark(f'loads{b}')
                # [128,2048] half loads; batch 0's first half loads in
                # quarters so the cast/transpose pipeline starts sooner
                x16 = {}
                for ct in range(CT):
                    x16[(ct, 0)] = x16_p.tile([P, HALF], f16, tag="x16", name=f"x_{b}_{ct}_0")
                    x16[(ct, 1)] = x16_p.tile([P, HALF], f16, tag="x16", name=f"x_{b}_{ct}_1")
                if fine_h0:
                    for nq in range(4):
                        for ct in range(CT):
                            nc.sync.dma_start(
                                out=x16[(ct, nq // 2)][:, (nq % 2) * NQ : (nq % 2 + 1) * NQ],
                                in_=x_d[b, ct * P : (ct + 1) * P, nq * NQ : (nq + 1) * NQ],
                            )
                else:
                    for h in range(2):
                        for ct in range(CT):
                            nc.sync.dma_start(
                                out=x16[(ct, h)][:],
                                in_=x_d[b, ct * P : (ct + 1) * P, h * HALF : (h + 1) * HALF],
                            )
                return x16

            def emit_casts(b, nq, split=False):
                _mark(f'cast{b}_{nq}')
                # fp16 -> fp8 casts for quarter nq split across ACT/DVE/Pool
                # (ACT takes ct0+ct3, DVE ct1, Pool ct2 to balance readiness)
                x16, f8q = state[b]["x16"], state[b]["f8q"]
                engs = {
                    0: nc.scalar.copy,
                    1: nc.gpsimd.tensor_copy,
                    2: nc.gpsimd.tensor_copy,
                    3: nc.vector.tensor_copy,
                }
                h, qo = nq // 2, (nq % 2) * NQ
                for ct in range(CT):
                    dst = f8q[ct // 2][:, ct % 2, nq * NQ : (nq + 1) * NQ]
                    src = x16[(ct, h)][:, qo : qo + NQ]
                    if split:
                        for hv in range(2):
                            engs[ct](
                                out=dst[:, hv * F : (hv + 1) * F],
                                in_=src[:, hv * F : (hv + 1) * F],
                            )
                    else:
                        engs[ct](out=dst, in_=src)

            def emit_tgroup(b, g, copy_eng=None):
                _mark(f'tg{b}_{g}')
                # transpose group g (2 k-tiles x 4 c-tiles = 8 fp8 PE
                # transposes into one PSUM bank). The hardware requires fp8
                # transpose outputs at element step 2, so xp and f1t keep
                # fp8 values in the low byte of 2-byte cells; the drain is
                # one dense bitcast-fp16 copy (2x DVE mode) and the S
                # matmuls read the even bytes via step-2 APs.
                f8q, f1t = state[b]["f8q"], state[b]["f1t"]
                h, gl = g // 8, g % 8
                xp = ps_mix.tile([P, 8, P, 2], f8, tag="mix", name=f"xp_{b}_{g}")
                for i in range(8):
                    kl, ct = gl * 2 + i // 4, i % 4
                    nc.tensor.transpose(
                        xp[:, i, :, 0],
                        f8q[ct // 2][
                            :, ct % 2, h * HALF + kl * P : h * HALF + (kl + 1) * P
                        ],
                        ident[:],
                    )
                dst = f1t[h][:, gl * 2 : gl * 2 + 2, :, :].bitcast(f16)
                if copy_eng is None:
                    copy_eng = nc.vector.tensor_copy if g % 2 == 0 else nc.scalar.copy
                copy_eng(out=dst, in_=xp[:].bitcast(f16))

            def emit_S_h0(b):
                _mark(f'Sh0_{b}')
                f1t, s_ps = state[b]["f1t"], state[b]["s_ps"]
                for q in range(8):
                    for m in range(CT):
                        nc.tensor.matmul(
                            s_ps[m][:],
                            lhsT=f1t[0][:, 2 * q : 2 * q + 2, m * P : (m + 1) * P, 0],
                            rhs=f1t[0][:, 2 * q : 2 * q + 2, :, 0],
                            start=(q == 0),
                            stop=False,
                            perf_mode=DR,
                        )

            def emit_S_h1_stats(b):
                _mark(f'Sh1_{b}')
                # m-outer so each negated row-min fires as its m-tile stops;
                # then the global-min chain (Pool/DVE) producing s0
                f1t, s_ps = state[b]["f1t"], state[b]["s_ps"]
                nm4 = soft_p.tile([P, CT], f32, tag="sm", name=f"nm4_{b}")
                for m in range(CT):
                    for q in range(8):
                        nc.tensor.matmul(
                            s_ps[m][:],
                            lhsT=f1t[1][:, 2 * q : 2 * q + 2, m * P : (m + 1) * P, 0],
                            rhs=f1t[1][:, 2 * q : 2 * q + 2, :, 0],
                            start=False,
                            stop=(q == 7),
                            perf_mode=DR,
                        )
                    nc.vector.tensor_reduce(
                        out=nm4[:, m : m + 1],
                        in_=s_ps[m][:],
                        axis=AX.X,
                        op=OP.min,
                        negate=True,
                    )
                nmx = soft_p.tile([P, 1], f32, tag="sm", name=f"nmx_{b}")
                nc.vector.tensor_reduce(out=nmx[:], in_=nm4[:], axis=AX.X, op=OP.max)
                s0n = soft_p.tile([P, 1], f32, tag="sm", name=f"s0n_{b}")
                nc.gpsimd.partition_all_reduce(
                    s0n[:], nmx[:], channels=P, reduce_op=bass_isa.ReduceOp.max
                )
                s0 = soft_p.tile([P, 1], f32, tag="sm", name=f"s0_{b}")
                s0i = nc.gpsimd.tensor_scalar_mul(out=s0[:], in0=s0n[:], scalar1=-1.0)
                state[b]["s0"] = s0
                state[b]["s0_ins"] = s0i.ins

            def emit_exps(b):
                _mark(f'exp{b}')
                # E = exp(s0 - S) in fp8 (symmetric); Zraw = fp32 rowsum accum
                s_ps, s0 = state[b]["s_ps"], state[b]["s0"]
                e2 = [
                    e2_p.tile([P, 2, F], f8, tag="e2", name=f"e2_{b}_{g}")
                    for g in range(2)
                ]
                z4 = soft_p.tile([P, CT], f32, tag="sm", name=f"z4_{b}")
                for m in range(CT):
                    nc.scalar.activation(
                        out=e2[m // 2][:, m % 2, :],
                        in_=s_ps[m][:],
                        func=AF.Exp,
                        bias=s0[:],
                        scale=-1.0,
                        accum_out=z4[:, m : m + 1],
                    )
                state[b]["e2"] = e2
                state[b]["z4"] = z4

            def emit_br(b):
                _mark(f'br{b}')
                # beta / Zraw, finite for any beta: clamp Zraw before the
                # reciprocal, and write br4 through a beta-predicated copy
                # so br4 is exactly 0.0 (not 0*NaN) for beta == 0 even if a
                # degenerate row overflowed the softmax normalizer
                z4 = state[b]["z4"]
                zs = soft_p.tile([P, CT], f32, tag="sm", name=f"zs_{b}")
                nc.gpsimd.tensor_scalar_max(out=zs[:], in0=z4[:], scalar1=1e-35)
                rz = soft_p.tile([P, CT], f32, tag="sm", name=f"rz_{b}")
                nc.vector.reciprocal(out=rz[:], in_=zs[:])
                rzb = soft_p.tile([P, CT], f32, tag="sm", name=f"rzb_{b}")
                nc.gpsimd.tensor_scalar_mul(out=rzb[:], in0=rz[:], scalar1=beta_sb[:])
                br4 = state[b]["br4"]
                nc.vector.copy_predicated(
                    out=br4[:],
                    mask=beta_nz[:].broadcast_to([P, CT]),
                    data=rzb[:],
                )

            def emit_warm_bridge(b, n):
                _mark(f'warm{b}')
                # keep the PE p-state hot through the softmax bubble with
                # dummy matmuls chained off f1t (run right after S)
                f1t = state[b]["f1t"]
                dum = ps_mix.tile([P, F], f32, tag="mix", name=f"dum_{b}")
                for i in range(n):
                    nc.tensor.matmul(
                        dum[:],
                        lhsT=f1t[1][:, 0:2, 0:P, 0],
                        rhs=f1t[1][:, 0:2, :, 0],
                        start=True,
                        stop=True,
                        perf_mode=DR,
                    )
                del dum

            def emit_casts_fine(b, h, engs):
                # [128,512]-piece casts, column-major so transpose group g
                # becomes ready after its 4 pieces; engs cycles per piece
                _mark(f'castf{b}_{h}')
                x16, f8q = state[b]["x16"], state[b]["f8q"]
                i = 0
                for q in range(4):
                    for ct in range(CT):
                        dst = f8q[ct // 2][:, ct % 2, h * HALF + q * F : h * HALF + (q + 1) * F]
                        src = x16[(ct, h)][:, q * F : (q + 1) * F]
                        engs[i % len(engs)](out=dst, in_=src)
                        i += 1

            def emit_fc_quarter(b, nq, cnt, last, ot):
                _mark(f'fc{b}_{nq}')
                # fc_raw = E @ f1 for quarter nq, m-outer; fused epilogue
                # y = (beta/Zraw)[c]*fc_raw + x
                x16, f8q = state[b]["x16"], state[b]["f8q"]
                e2, br4 = state[b]["e2"], state[b]["br4"]
                h, qo = nq // 2, (nq % 2) * NQ
                wide = last  # last batch rotates mix+s (8 banks), uses pairs
                for m in range(CT):
                    if nq % 2 == 0:
                        ot[(m, h)] = out_p.tile(
                            [P, HALF], f16, tag="out", name=f"ot_{b}_{h}_{m}"
                        )
                    o = ot[(m, h)]
                    for jj in range(2):
                        j = nq * 2 + jj
                        pool = ps_s if (wide and cnt[0] % 2 == 1) else ps_mix
                        tag = "s" if pool is ps_s else "mix"
                        f_ps = pool.tile([P, F], f32, tag=tag, name=f"f_{b}_{j}_{m}")
                        for qe in range(2):
                            nc.tensor.matmul(
                                f_ps[:],
                                lhsT=e2[qe][:, :, m * P : (m + 1) * P],
                                rhs=f8q[qe][:, :, j * F : (j + 1) * F],
                                start=(qe == 0),
                                stop=(qe == 1),
                                perf_mode=DR,
                            )
                        oslice = o[:, qo + jj * F : qo + (jj + 1) * F]
                        xslice = x16[(m, h)][:, qo + jj * F : qo + (jj + 1) * F]
                        c = cnt[0]
                        if b == 0 and nq == 1:
                            # DVE-light quarter: the b1 row-min rail owns DVE
                            kind = (1, 2, 0, 2)[c % 4]
                        else:
                            kind = (0, 2, 0, 1)[c % 4]
                        if kind:
                            # PSUM can only be read by ACT/DVE: ACT scales
                            # fc out of PSUM, the fp16 residual add runs on
                            # DVE (2x) or Pool (all-SBUF)
                            tmp = tmp_p.tile([P, F], f16, tag="tmp", name=f"tp_{b}_{j}_{m}")
                            nc.scalar.mul(out=tmp[:], in_=f_ps[:], mul=br4[:, m : m + 1])
                            aeng = nc.vector if kind == 1 else nc.gpsimd
                            aeng.tensor_tensor(
                                out=oslice, in0=tmp[:], in1=xslice, op=OP.add
                            )
                        else:
                            nc.vector.scalar_tensor_tensor(
                                out=oslice,
                                in0=f_ps[:],
                                scalar=br4[:, m : m + 1],
                                in1=xslice,
                                op0=OP.mult,
                                op1=OP.add,
                            )
                        del f_ps
                        cnt[0] += 1
                    nc.sync.dma_start(
                        out=y_d[b, m * P : (m + 1) * P, nq * NQ : (nq + 1) * NQ],
                        in_=o[:, qo : qo + NQ],
                    )

            def new_state(b):
                state[b] = {
                    "f8q": [
                        f8_p.tile([P, 2, HW], f8, tag="f8", name=f"f8_{b}_{q}")
                        for q in range(2)
                    ],
                    "f1t": [
                        f1t_p.tile([P, KTH, F, 2], f8, tag="f1t", name=f"f1t_{b}_{h}")
                        for h in range(2)
                    ],
                    "s_ps": [
                        ps_s.tile([P, F], f32, tag="s", name=f"s_ps_{b}_{m}")
                        for m in range(CT)
                    ],
                    "br4": soft_p.tile([P, CT], f32, tag="sm", name=f"br4_{b}"),
                }
                nc.gpsimd.memset(state[b]["br4"][:], 0.0)

            # ---- batch 0 prep: loads, transposes, S, casts interleaved ----
            # (transposes read x16 directly, so only loads gate them; the
            # fp8 casts feed the fc rhs and can lag)
            new_state(0)
            state[0]["x16"] = emit_loads(0, fine_h0=True)
            # column-major [128,512] cast pieces: transpose group g unblocks
            # after its quarter's four pieces
            emit_casts_fine(
                0, 0,
                [nc.scalar.copy, nc.vector.tensor_copy,
                 nc.gpsimd.tensor_copy, nc.vector.tensor_copy],
            )
            for g in range(8):
                emit_tgroup(0, g)
            emit_S_h0(0)
            emit_casts_fine(
                0, 1,
                [nc.scalar.copy, nc.vector.tensor_copy,
                 nc.gpsimd.tensor_copy, nc.vector.tensor_copy],
            )
            for g in range(8, 16):
                emit_tgroup(0, g)
            emit_S_h1_stats(0)

            # ---- interleaved b0 fc / b1 chain, with b1-chain priority so
            # ---- the terminal S(1)->exp(1)->fc(1)->stores chain starts ASAP
            assert BL == 2
            new_state(1)
            state[1]["x16"] = emit_loads(1, fine_h0=False)
            # b1 h0 casts ahead of exps(0) on DVE/Pool (ACT owns the exps)
            emit_casts_fine(
                1, 0,
                [nc.vector.tensor_copy, nc.gpsimd.tensor_copy,
                 nc.vector.tensor_copy, nc.gpsimd.tensor_copy],
            )
            for g in range(8):
                emit_tgroup(1, g)
            emit_S_h0(1)
            emit_exps(0)
            emit_br(0)
            emit_casts_fine(
                1, 1,
                [nc.scalar.copy, nc.vector.tensor_copy,
                 nc.gpsimd.tensor_copy, nc.vector.tensor_copy],
            )
            for g in range(8, 16):
                emit_tgroup(1, g)
            emit_S_h1_stats(1)
            cnt0, ot0 = [0], {}
            emit_fc_quarter(0, 0, cnt0, False, ot0)
            emit_fc_quarter(0, 1, cnt0, False, ot0)
            emit_fc_quarter(0, 2, cnt0, False, ot0)
            emit_exps(1)
            emit_br(1)
            emit_fc_quarter(0, 3, cnt0, False, ot0)
            cnt1, ot1 = [0], {}
            for nq in range(4):
                emit_fc_quarter(1, nq, cnt1, True, ot1)
    nc.finalize()
    return nc


def _get_nc():
    if "nc" not in _CACHE:
        _CACHE["nc"] = _build()
    return _CACHE["nc"]


def kernel(x: np.ndarray, beta: np.ndarray, **kw) -> np.ndarray:
    from concourse.bass_utils import run_bass_kernel_spmd

    x = np.asarray(x)
    beta = np.ascontiguousarray(np.asarray(beta, dtype=np.float32))
    assert x.shape == (B, C, 64, 64), x.shape

    x16 = np.ascontiguousarray(x.reshape(B, C, HW).astype(np.float16))
    in_maps = [
        {"x": np.ascontiguousarray(x16[i * BL : (i + 1) * BL]), "beta": beta}
        for i in range(NCORES)
    ]
    nc = _get_nc()
    res = run_bass_kernel_spmd(nc, in_maps, core_ids=list(range(NCORES)))
    out = np.concatenate([r["y"] for r in res.results], axis=0)
    return out.reshape(B, C, 64, 64).astype(np.float32)

